# revision 9
# baseline (speedup 1.0000x reference)
"""MoE layer (top-2 of 8 experts, SwiGLU) on 8 Trainium2 NeuronCores.

Strategy (expert-parallel, per sharding hint):
  - Host computes the gate routing (logits -> top-2 ids/gates, fp32) and the
    load-balance loss; this is the control plane (~0.05% of total FLOPs).
  - Tokens are dispatched (gathered) per expert on the host; core c receives
    the tokens routed to expert c, pre-transposed to [d, tokens] layout, plus
    expert c's weights in lhsT tile layout (bf16).
  - Each core runs the grouped SwiGLU GEMM chain on the TensorEngine:
        h1T = w1.T @ xT ; h2T = w2.T @ xT       (PSUM fp32 accumulation)
        hsT = (h1T + b1) * silu(h2T + b2)        (ACT + DVE)
        yT  = wp.T @ hsT                          (PSUM fp32 accumulation)
  - Host combines: each token appears in exactly K=2 expert outputs; the full
    output is the gate-weighted sum of its two gathered contributions.
"""

import os
import numpy as np
import ml_dtypes

import concourse.bass as bass
import concourse.mybir as mybir
import concourse.tile as tile
from concourse import bacc
from concourse.bass_utils import run_bass_kernel_spmd

B, S, D, H, E, K = 4, 2048, 768, 3072, 8, 2
LB_SCALE = 0.01
NOISY_STD = 1.0
P = 128
DK = D // P    # 6
HK = H // P    # 24
N_CORES = 8

BF16 = mybir.dt.bfloat16
F32 = mybir.dt.float32

LAST_EXEC_NS = None  # stashed by kernel() when MOE_TRACE=1
LAST_TRACE = None


def _token_tiles(cap):
    """Split cap (multiple of 128) into tiles of <=512, remainder last
    (N=128 tiles are LDWEIGHTS-bound; keep them out of the HAM-cold start)."""
    # First tile 256 tokens: halves the critical first DMA the PE waits on
    # while staying above the N>=256 LDWEIGHTS-hiding threshold.
    tiles = []
    t0 = 0
    if cap > 512:
        tiles.append((0, 256))
        t0 = 256
    while t0 < cap:
        nt = min(512, cap - t0)
        tiles.append((t0, nt))
        t0 += nt
    # A trailing tile below 256 tokens is LDWEIGHTS-bound on the PE (weight
    # load 107ns > N*0.42ns stream); rebalance the last two tiles.
    if len(tiles) >= 2 and tiles[-1][1] < 256:
        t_prev, n_prev = tiles[-2]
        total = n_prev + tiles[-1][1]
        n1 = total // 2
        tiles[-2] = (t_prev, n1)
        tiles[-1] = (t_prev + n1, total - n1)
    return tiles


def _build_nc(cap, has_b1, has_b2):
    nc = bacc.Bacc("TRN2", target_bir_lowering=False, num_devices=N_CORES)

    xgt_d = nc.dram_tensor("xgt", [P, DK, cap], BF16, kind="ExternalInput")
    w1_d = nc.dram_tensor("w1", [P, DK, H], BF16, kind="ExternalInput")
    w2_d = nc.dram_tensor("w2", [P, DK, H], BF16, kind="ExternalInput")
    wp_d = nc.dram_tensor("wp", [P, HK, D], BF16, kind="ExternalInput")
    b1_d = nc.dram_tensor("b1", [P, HK], F32, kind="ExternalInput")
    b2_d = nc.dram_tensor("b2", [P, HK], F32, kind="ExternalInput")
    yt_d = nc.dram_tensor("yt", [P, DK, cap], F32, kind="ExternalOutput")

    ttiles = _token_tiles(cap)

    with tile.TileContext(nc) as tc:
        with (
            tc.tile_pool(name="wres", bufs=1) as wres,
            tc.tile_pool(name="hsp", bufs=1) as hsp,
            tc.tile_pool(name="silup", bufs=3) as silup,
            tc.tile_pool(name="h1sp", bufs=3) as h1sp,
            tc.tile_pool(name="ytsb", bufs=3) as ytsb,
            tc.tile_pool(name="ps", bufs=2, space="PSUM") as ps,
            tc.tile_pool(name="ps2", bufs=2, space="PSUM") as ps2,
        ):
            # ---- resident tiles ----
            w1_t = wres.tile([P, DK, H], BF16)
            w2_t = wres.tile([P, DK, H], BF16)
            wp_t = wres.tile([P, HK, D], BF16)
            xgt_t = wres.tile([P, DK, cap], BF16)
            b1_t = wres.tile([P, HK], F32)
            b2_t = wres.tile([P, HK], F32)
            # DMA issue costs ~0.6us/instr on a sequencer; spread the
            # critical first-tile loads across four sequencers so the PE can
            # start as early as possible, then stream the rest on sync/gpsimd.
            issuers = [nc.sync, nc.gpsimd, nc.scalar]
            crit = []
            t0_0, nt_0 = ttiles[0]
            HG = 4  # hk chunk for weight streaming
            hs0 = slice(0, P)          # hk=0 only: first matmul group
            for dk in range(DK):
                crit.append((xgt_t[:, dk, t0_0:t0_0 + nt_0],
                             xgt_d[:, dk, t0_0:t0_0 + nt_0]))
                crit.append((w2_t[:, dk, hs0], w2_d[:, dk, hs0]))
                crit.append((w1_t[:, dk, hs0], w1_d[:, dk, hs0]))
            for i, (dst, srcap) in enumerate(crit):
                issuers[i % 3].dma_start(dst, srcap)
            rest = []
            hs13 = slice(P, HG * P)    # hk 1..3
            for dk in range(DK):
                rest.append((w2_t[:, dk, hs13], w2_d[:, dk, hs13]))
                rest.append((w1_t[:, dk, hs13], w1_d[:, dk, hs13]))
            for hk in range(HG, HK, HG):
                hs = slice(hk * P, (hk + HG) * P)
                for dk in range(DK):
                    rest.append((w2_t[:, dk, hs], w2_d[:, dk, hs]))
                    rest.append((w1_t[:, dk, hs], w1_d[:, dk, hs]))
            for (t0, nt) in ttiles[1:]:
                for dk in range(DK):
                    rest.append((xgt_t[:, dk, t0:t0 + nt],
                                 xgt_d[:, dk, t0:t0 + nt]))
            for hk in range(0, HK, HG):
                rest.append((wp_t[:, hk:hk + HG], wp_d[:, hk:hk + HG]))
            for i, (dst, srcap) in enumerate(rest):
                (nc.sync if i % 2 == 0 else nc.gpsimd).dma_start(dst, srcap)
            nc.sync.dma_start(b1_t[:], b1_d[:])
            nc.gpsimd.dma_start(b2_t[:], b2_d[:])

            for (t0, nt) in ttiles:
                hst = hsp.tile([P, HK, 512], BF16, name="hst")
                # ---- first GEMMs + SwiGLU ----
                for hk in range(HK):
                    h1p = ps.tile([P, 512], F32, name="h1p")
                    h2p = ps.tile([P, 512], F32, name="h2p")
                    for dk in range(DK):
                        nc.tensor.matmul(
                            h2p[:, :nt],
                            w2_t[:, dk, hk * P:(hk + 1) * P],
                            xgt_t[:, dk, t0:t0 + nt],
                            start=(dk == 0), stop=(dk == DK - 1),
                        )
                    for dk in range(DK):
                        nc.tensor.matmul(
                            h1p[:, :nt],
                            w1_t[:, dk, hk * P:(hk + 1) * P],
                            xgt_t[:, dk, t0:t0 + nt],
                            start=(dk == 0), stop=(dk == DK - 1),
                        )
                    s_t = silup.tile([P, 512], F32, name="s_t")
                    nc.scalar.activation(
                        s_t[:, :nt], h2p[:, :nt],
                        mybir.ActivationFunctionType.Silu,
                        bias=(b2_t[:, hk:hk + 1] if has_b2 else 0.0),
                    )
                    if has_b1:
                        h1s = h1sp.tile([P, 512], F32, name="h1s")
                        nc.vector.tensor_scalar_add(
                            h1s[:, :nt], h1p[:, :nt], b1_t[:, hk:hk + 1]
                        )
                        mul_in = h1s
                    else:
                        mul_in = h1p
                    nc.vector.tensor_mul(
                        out=hst[:, hk, :nt], in0=mul_in[:, :nt], in1=s_t[:, :nt]
                    )

                # ---- second GEMM, store transposed; host finishes ----
                for do in range(DK):
                    yp = ps2.tile([P, 512], F32, name="yp")
                    for hk in range(HK):
                        nc.tensor.matmul(
                            yp[:, :nt],
                            wp_t[:, hk, do * P:(do + 1) * P],
                            hst[:, hk, :nt],
                            start=(hk == 0), stop=(hk == HK - 1),
                        )
                    yo = ytsb.tile([P, 512], F32, name="yo")
                    nc.vector.tensor_copy(yo[:, :nt], yp[:, :nt])
                    nc.sync.dma_start(yt_d[:, do, t0:t0 + nt], yo[:, :nt])

    nc.compile()
    return nc


def kernel(x, gate_w, noise_weight, noise, w1, b1, w2, b2, wp, bp):
    global LAST_EXEC_NS, LAST_TRACE

    x = np.asarray(x, dtype=np.float32)
    gate_w = np.asarray(gate_w, dtype=np.float32)
    noise_weight = np.asarray(noise_weight, dtype=np.float32)
    noise = np.asarray(noise, dtype=np.float32)
    w1 = np.asarray(w1, dtype=np.float32)
    b1 = np.asarray(b1, dtype=np.float32)
    w2 = np.asarray(w2, dtype=np.float32)
    b2 = np.asarray(b2, dtype=np.float32)
    wp = np.asarray(wp, dtype=np.float32)
    bp = np.asarray(bp, dtype=np.float32)

    T = B * S
    xf = x.reshape(T, D)

    # ---------- host routing (control plane, fp32) ----------
    logits = xf @ gate_w.T                                   # [T, E]
    ln = logits
    if np.any(noise_weight):
        ln = logits + noise.reshape(T, E) * (NOISY_STD * noise_weight)
    part = np.partition(ln, E - 2, axis=1)
    t2 = part[:, E - 2]
    t1 = part[:, E - 1]
    mask = ln >= t2[:, None]                                 # top-2 set
    nsel = mask.sum(1)
    if np.any(nsel != K):  # tie fallback: exact top-k by sort
        order = np.argsort(-ln, axis=1, kind="stable")
        mask = np.zeros_like(mask)
        np.put_along_axis(mask, order[:, :K], True, axis=1)
    e_all = np.exp(ln - t1[:, None], dtype=np.float32)
    denom = 1.0 + np.exp(t2 - t1, dtype=np.float32)
    gates = np.where(mask, e_all / denom[:, None], 0.0).astype(np.float32)

    # load-balance loss from clean logits (full softmax)
    lmax = logits.max(1, keepdims=True)
    sm = np.exp(logits - lmax, dtype=np.float32)
    sm /= sm.sum(1, keepdims=True)
    gwm = sm.mean(0, dtype=np.float32)
    lb = np.float32(np.mean((gwm - 1.0 / E) ** 2, dtype=np.float32) * LB_SCALE)

    # ---------- dispatch: gather tokens per expert ----------
    idxs = [np.nonzero(mask[:, e])[0] for e in range(E)]
    counts = np.array([len(i) for i in idxs])
    cap = int(counts.max())  # matmul free dim needs no alignment

    has_b1 = bool(np.any(b1))
    has_b2 = bool(np.any(b2))

    in_maps = []
    for e in range(E):
        idx = idxs[e]
        n_e = len(idx)
        xg = np.zeros((cap, D), dtype=np.float32)
        xg[:n_e] = xf[idx]
        # [cap, D] -> [P, DK, cap] with element (p, dk, s) = xg[s, dk*128+p]
        xgt = np.ascontiguousarray(
            xg.reshape(cap, DK, P).transpose(2, 1, 0)
        ).astype(ml_dtypes.bfloat16)
        w1e = np.ascontiguousarray(
            w1[e].reshape(DK, P, H).transpose(1, 0, 2)
        ).astype(ml_dtypes.bfloat16)                          # [P, DK, H]
        w2e = np.ascontiguousarray(
            w2[e].reshape(DK, P, H).transpose(1, 0, 2)
        ).astype(ml_dtypes.bfloat16)
        wpe = np.ascontiguousarray(
            wp[e].reshape(HK, P, D).transpose(1, 0, 2)
        ).astype(ml_dtypes.bfloat16)                          # [P, HK, D]
        in_maps.append({
            "xgt": xgt, "w1": w1e, "w2": w2e, "wp": wpe,
            "b1": np.ascontiguousarray(b1[e].reshape(HK, P).T),
            "b2": np.ascontiguousarray(b2[e].reshape(HK, P).T),
        })

    # ---------- device: grouped SwiGLU GEMMs on 8 cores ----------
    nc = _build_nc(cap, has_b1, has_b2)
    trace = os.environ.get("MOE_TRACE") == "1"
    out = run_bass_kernel_spmd(
        nc, in_maps, core_ids=list(range(N_CORES)), trace=trace,
    )
    LAST_EXEC_NS = out.exec_time_ns
    LAST_TRACE = out.instructions_and_trace[1] if out.instructions_and_trace else None

    # ---------- combine on host ----------
    # Per-expert output yt [P, DK, cap] -> token-major [cap, D], + bp, * gate.
    # Each token has exactly K=2 contributions; gather-sum them.
    allout = np.empty((E * cap, D), dtype=np.float32)
    g_all = np.zeros(E * cap, dtype=np.float32)
    for e in range(E):
        yt = out.results[e]["yt"]                             # [P, DK, cap]
        allout[e * cap:(e + 1) * cap] = (
            yt.transpose(2, 1, 0).reshape(cap, D) + bp[e]
        )
        g_all[e * cap:e * cap + len(idxs[e])] = gates[idxs[e], e]
    allout *= g_all[:, None]

    pos = np.zeros((T, K), dtype=np.int64)
    cnt = np.zeros(T, dtype=np.int64)
    for e in range(E):
        idx = idxs[e]
        pos[idx, cnt[idx]] = e * cap + np.arange(len(idx))
        cnt[idx] += 1
    assert np.all(cnt == K)
    y = allout[pos[:, 0]] + allout[pos[:, 1]]
    return y.reshape(B, S, D).astype(np.float32), lb


# revision 10
# speedup vs baseline: 1.1970x; 1.1970x over previous
"""MoE layer (top-2 of 8 experts, SwiGLU) on 8 Trainium2 NeuronCores.

Strategy (expert-parallel, per sharding hint):
  - Host computes the gate routing (logits -> top-2 ids/gates, fp32) and the
    load-balance loss; this is the control plane (~0.05% of total FLOPs).
  - Tokens are dispatched (gathered) per expert on the host; core c receives
    the tokens routed to expert c, pre-transposed to [d, tokens] layout, plus
    expert c's weights in lhsT tile layout (bf16).
  - Each core runs the grouped SwiGLU GEMM chain on the TensorEngine:
        h1T = w1.T @ xT ; h2T = w2.T @ xT       (PSUM fp32 accumulation)
        hsT = (h1T + b1) * silu(h2T + b2)        (ACT + DVE)
        yT  = wp.T @ hsT                          (PSUM fp32 accumulation)
  - Host combines: each token appears in exactly K=2 expert outputs; the full
    output is the gate-weighted sum of its two gathered contributions.
"""

import os
import numpy as np
import ml_dtypes

import concourse.bass as bass
import concourse.mybir as mybir
import concourse.tile as tile
from concourse import bacc
from concourse.bass_utils import run_bass_kernel_spmd

B, S, D, H, E, K = 4, 2048, 768, 3072, 8, 2
LB_SCALE = 0.01
NOISY_STD = 1.0
P = 128
DK = D // P    # 6
HK = H // P    # 24
N_CORES = 8

BF16 = mybir.dt.bfloat16
F32 = mybir.dt.float32

LAST_EXEC_NS = None  # stashed by kernel() when MOE_TRACE=1
LAST_TRACE = None


def _token_tiles(cap):
    """Split cap (multiple of 128) into tiles of <=512, remainder last
    (N=128 tiles are LDWEIGHTS-bound; keep them out of the HAM-cold start)."""
    tiles = []
    t0 = 0
    while t0 < cap:
        nt = min(512, cap - t0)
        tiles.append((t0, nt))
        t0 += nt
    # A trailing tile below 256 tokens is LDWEIGHTS-bound on the PE (weight
    # load 107ns > N*0.42ns stream); rebalance the last two tiles so both
    # are >=256 (multiples of 128).
    if len(tiles) >= 2 and tiles[-1][1] < 256:
        t_prev, n_prev = tiles[-2]
        n_last = tiles[-1][1]
        total = n_prev + n_last
        n1 = (total // 2 + 127) // 128 * 128
        n2 = total - n1
        tiles[-2] = (t_prev, n1)
        tiles[-1] = (t_prev + n1, n2)
    return tiles


def _build_nc(cap, has_b1, has_b2):
    nc = bacc.Bacc("TRN2", target_bir_lowering=False, num_devices=N_CORES)

    xgt_d = nc.dram_tensor("xgt", [P, DK, cap], BF16, kind="ExternalInput")
    w1_d = nc.dram_tensor("w1", [P, DK, H], BF16, kind="ExternalInput")
    w2_d = nc.dram_tensor("w2", [P, DK, H], BF16, kind="ExternalInput")
    wp_d = nc.dram_tensor("wp", [P, HK, D], BF16, kind="ExternalInput")
    b1_d = nc.dram_tensor("b1", [P, HK], F32, kind="ExternalInput")
    b2_d = nc.dram_tensor("b2", [P, HK], F32, kind="ExternalInput")
    yt_d = nc.dram_tensor("yt", [P, DK, cap], F32, kind="ExternalOutput")

    ttiles = _token_tiles(cap)

    with tile.TileContext(nc) as tc:
        with (
            tc.tile_pool(name="wres", bufs=1) as wres,
            tc.tile_pool(name="hsp", bufs=1) as hsp,
            tc.tile_pool(name="silup", bufs=3) as silup,
            tc.tile_pool(name="h1sp", bufs=3) as h1sp,
            tc.tile_pool(name="ytsb", bufs=3) as ytsb,
            tc.tile_pool(name="ps", bufs=2, space="PSUM") as ps,
            tc.tile_pool(name="ps2", bufs=2, space="PSUM") as ps2,
        ):
            # ---- resident tiles ----
            w1_t = wres.tile([P, DK, H], BF16)
            w2_t = wres.tile([P, DK, H], BF16)
            wp_t = wres.tile([P, HK, D], BF16)
            xgt_t = wres.tile([P, DK, cap], BF16)
            b1_t = wres.tile([P, HK], F32)
            b2_t = wres.tile([P, HK], F32)
            # DMA issue costs ~0.6us/instr on a sequencer; spread the
            # critical first-tile loads across four sequencers so the PE can
            # start as early as possible, then stream the rest on sync/gpsimd.
            issuers = [nc.sync, nc.gpsimd, nc.scalar]
            crit = []
            t0_0, nt_0 = ttiles[0]
            HG = 4  # hk chunk for weight streaming
            hs0 = slice(0, HG * P)
            for dk in range(DK):
                crit.append((xgt_t[:, dk, t0_0:t0_0 + nt_0],
                             xgt_d[:, dk, t0_0:t0_0 + nt_0]))
            for dk in range(DK):
                crit.append((w2_t[:, dk, hs0], w2_d[:, dk, hs0]))
            for dk in range(DK):
                crit.append((w1_t[:, dk, hs0], w1_d[:, dk, hs0]))
            for i, (dst, srcap) in enumerate(crit):
                issuers[i % 3].dma_start(dst, srcap)
            rest = []
            for hk in range(HG, HK, HG):
                hs = slice(hk * P, (hk + HG) * P)
                for dk in range(DK):
                    rest.append((w2_t[:, dk, hs], w2_d[:, dk, hs]))
                    rest.append((w1_t[:, dk, hs], w1_d[:, dk, hs]))
            for (t0, nt) in ttiles[1:]:
                for dk in range(DK):
                    rest.append((xgt_t[:, dk, t0:t0 + nt],
                                 xgt_d[:, dk, t0:t0 + nt]))
            for hk in range(0, HK, HG):
                rest.append((wp_t[:, hk:hk + HG], wp_d[:, hk:hk + HG]))
            for i, (dst, srcap) in enumerate(rest):
                (nc.sync if i % 2 == 0 else nc.gpsimd).dma_start(dst, srcap)
            nc.sync.dma_start(b1_t[:], b1_d[:])
            nc.gpsimd.dma_start(b2_t[:], b2_d[:])

            for (t0, nt) in ttiles:
                hst = hsp.tile([P, HK, 512], BF16, name="hst")
                # ---- first GEMMs + SwiGLU ----
                for hk in range(HK):
                    h1p = ps.tile([P, 512], F32, name="h1p")
                    h2p = ps.tile([P, 512], F32, name="h2p")
                    for dk in range(DK):
                        nc.tensor.matmul(
                            h2p[:, :nt],
                            w2_t[:, dk, hk * P:(hk + 1) * P],
                            xgt_t[:, dk, t0:t0 + nt],
                            start=(dk == 0), stop=(dk == DK - 1),
                        )
                    for dk in range(DK):
                        nc.tensor.matmul(
                            h1p[:, :nt],
                            w1_t[:, dk, hk * P:(hk + 1) * P],
                            xgt_t[:, dk, t0:t0 + nt],
                            start=(dk == 0), stop=(dk == DK - 1),
                        )
                    s_t = silup.tile([P, 512], F32, name="s_t")
                    nc.scalar.activation(
                        s_t[:, :nt], h2p[:, :nt],
                        mybir.ActivationFunctionType.Silu,
                        bias=(b2_t[:, hk:hk + 1] if has_b2 else 0.0),
                    )
                    if has_b1:
                        h1s = h1sp.tile([P, 512], F32, name="h1s")
                        nc.vector.tensor_scalar_add(
                            h1s[:, :nt], h1p[:, :nt], b1_t[:, hk:hk + 1]
                        )
                        mul_in = h1s
                    else:
                        mul_in = h1p
                    nc.vector.tensor_mul(
                        out=hst[:, hk, :nt], in0=mul_in[:, :nt], in1=s_t[:, :nt]
                    )

                # ---- second GEMM, store transposed; host finishes ----
                for do in range(DK):
                    yp = ps2.tile([P, 512], F32, name="yp")
                    for hk in range(HK):
                        nc.tensor.matmul(
                            yp[:, :nt],
                            wp_t[:, hk, do * P:(do + 1) * P],
                            hst[:, hk, :nt],
                            start=(hk == 0), stop=(hk == HK - 1),
                        )
                    yo = ytsb.tile([P, 512], F32, name="yo")
                    nc.vector.tensor_copy(yo[:, :nt], yp[:, :nt])
                    nc.sync.dma_start(yt_d[:, do, t0:t0 + nt], yo[:, :nt])

    nc.compile()
    return nc


def kernel(x, gate_w, noise_weight, noise, w1, b1, w2, b2, wp, bp):
    global LAST_EXEC_NS, LAST_TRACE

    x = np.asarray(x, dtype=np.float32)
    gate_w = np.asarray(gate_w, dtype=np.float32)
    noise_weight = np.asarray(noise_weight, dtype=np.float32)
    noise = np.asarray(noise, dtype=np.float32)
    w1 = np.asarray(w1, dtype=np.float32)
    b1 = np.asarray(b1, dtype=np.float32)
    w2 = np.asarray(w2, dtype=np.float32)
    b2 = np.asarray(b2, dtype=np.float32)
    wp = np.asarray(wp, dtype=np.float32)
    bp = np.asarray(bp, dtype=np.float32)

    T = B * S
    xf = x.reshape(T, D)

    # ---------- host routing (control plane, fp32) ----------
    logits = xf @ gate_w.T                                   # [T, E]
    ln = logits
    if np.any(noise_weight):
        ln = logits + noise.reshape(T, E) * (NOISY_STD * noise_weight)
    part = np.partition(ln, E - 2, axis=1)
    t2 = part[:, E - 2]
    t1 = part[:, E - 1]
    mask = ln >= t2[:, None]                                 # top-2 set
    nsel = mask.sum(1)
    if np.any(nsel != K):  # tie fallback: exact top-k by sort
        order = np.argsort(-ln, axis=1, kind="stable")
        mask = np.zeros_like(mask)
        np.put_along_axis(mask, order[:, :K], True, axis=1)
    e_all = np.exp(ln - t1[:, None], dtype=np.float32)
    denom = 1.0 + np.exp(t2 - t1, dtype=np.float32)
    gates = np.where(mask, e_all / denom[:, None], 0.0).astype(np.float32)

    # load-balance loss from clean logits (full softmax)
    lmax = logits.max(1, keepdims=True)
    sm = np.exp(logits - lmax, dtype=np.float32)
    sm /= sm.sum(1, keepdims=True)
    gwm = sm.mean(0, dtype=np.float32)
    lb = np.float32(np.mean((gwm - 1.0 / E) ** 2, dtype=np.float32) * LB_SCALE)

    # ---------- dispatch: gather tokens per expert ----------
    idxs = [np.nonzero(mask[:, e])[0] for e in range(E)]
    counts = np.array([len(i) for i in idxs])
    cap = int(np.ceil(counts.max() / P) * P)

    has_b1 = bool(np.any(b1))
    has_b2 = bool(np.any(b2))

    in_maps = []
    for e in range(E):
        idx = idxs[e]
        n_e = len(idx)
        xg = np.zeros((cap, D), dtype=np.float32)
        xg[:n_e] = xf[idx]
        # [cap, D] -> [P, DK, cap] with element (p, dk, s) = xg[s, dk*128+p]
        xgt = np.ascontiguousarray(
            xg.reshape(cap, DK, P).transpose(2, 1, 0)
        ).astype(ml_dtypes.bfloat16)
        w1e = np.ascontiguousarray(
            w1[e].reshape(DK, P, H).transpose(1, 0, 2)
        ).astype(ml_dtypes.bfloat16)                          # [P, DK, H]
        w2e = np.ascontiguousarray(
            w2[e].reshape(DK, P, H).transpose(1, 0, 2)
        ).astype(ml_dtypes.bfloat16)
        wpe = np.ascontiguousarray(
            wp[e].reshape(HK, P, D).transpose(1, 0, 2)
        ).astype(ml_dtypes.bfloat16)                          # [P, HK, D]
        in_maps.append({
            "xgt": xgt, "w1": w1e, "w2": w2e, "wp": wpe,
            "b1": np.ascontiguousarray(b1[e].reshape(HK, P).T),
            "b2": np.ascontiguousarray(b2[e].reshape(HK, P).T),
        })

    # ---------- device: grouped SwiGLU GEMMs on 8 cores ----------
    nc = _build_nc(cap, has_b1, has_b2)
    trace = os.environ.get("MOE_TRACE") == "1"
    out = run_bass_kernel_spmd(
        nc, in_maps, core_ids=list(range(N_CORES)), trace=trace,
    )
    LAST_EXEC_NS = out.exec_time_ns
    LAST_TRACE = out.instructions_and_trace[1] if out.instructions_and_trace else None

    # ---------- combine on host ----------
    # Per-expert output yt [P, DK, cap] -> token-major [cap, D], + bp, * gate.
    # Each token has exactly K=2 contributions; gather-sum them.
    allout = np.empty((E * cap, D), dtype=np.float32)
    g_all = np.zeros(E * cap, dtype=np.float32)
    for e in range(E):
        yt = out.results[e]["yt"]                             # [P, DK, cap]
        allout[e * cap:(e + 1) * cap] = (
            yt.transpose(2, 1, 0).reshape(cap, D) + bp[e]
        )
        g_all[e * cap:e * cap + len(idxs[e])] = gates[idxs[e], e]
    allout *= g_all[:, None]

    pos = np.zeros((T, K), dtype=np.int64)
    cnt = np.zeros(T, dtype=np.int64)
    for e in range(E):
        idx = idxs[e]
        pos[idx, cnt[idx]] = e * cap + np.arange(len(idx))
        cnt[idx] += 1
    assert np.all(cnt == K)
    y = allout[pos[:, 0]] + allout[pos[:, 1]]
    return y.reshape(B, S, D).astype(np.float32), lb


# revision 11
# speedup vs baseline: 1.1987x; 1.0015x over previous
"""MoE layer (top-2 of 8 experts, SwiGLU) on 8 Trainium2 NeuronCores.

Strategy (expert-parallel, per sharding hint):
  - Host computes the gate routing (logits -> top-2 ids/gates, fp32) and the
    load-balance loss; this is the control plane (~0.05% of total FLOPs).
  - Tokens are dispatched (gathered) per expert on the host; core c receives
    the tokens routed to expert c, pre-transposed to [d, tokens] layout, plus
    expert c's weights in lhsT tile layout (bf16).
  - Each core runs the grouped SwiGLU GEMM chain on the TensorEngine:
        h1T = w1.T @ xT ; h2T = w2.T @ xT       (PSUM fp32 accumulation)
        hsT = (h1T + b1) * silu(h2T + b2)        (ACT + DVE)
        yT  = wp.T @ hsT                          (PSUM fp32 accumulation)
  - Host combines: each token appears in exactly K=2 expert outputs; the full
    output is the gate-weighted sum of its two gathered contributions.
"""

import os
import numpy as np
import ml_dtypes

import concourse.bass as bass
import concourse.mybir as mybir
import concourse.tile as tile
from concourse import bacc
from concourse.bass_utils import run_bass_kernel_spmd

B, S, D, H, E, K = 4, 2048, 768, 3072, 8, 2
LB_SCALE = 0.01
NOISY_STD = 1.0
P = 128
DK = D // P    # 6
HK = H // P    # 24
N_CORES = 8

BF16 = mybir.dt.bfloat16
F32 = mybir.dt.float32

LAST_EXEC_NS = None  # stashed by kernel() when MOE_TRACE=1
LAST_TRACE = None
LAST_PER_CORE = None
_NC_CACHE = {}  # (cap, has_b1, has_b2) -> compiled Bass program


def _token_tiles(cap):
    """Split cap (multiple of 128) into tiles of <=512, remainder last
    (N=128 tiles are LDWEIGHTS-bound; keep them out of the HAM-cold start)."""
    tiles = []
    t0 = 0
    while t0 < cap:
        nt = min(512, cap - t0)
        tiles.append((t0, nt))
        t0 += nt
    # A trailing tile below 256 tokens is LDWEIGHTS-bound on the PE (weight
    # load 107ns > N*0.42ns stream); rebalance the last two tiles so both
    # are >=256 (multiples of 128).
    if len(tiles) >= 2 and tiles[-1][1] < 256:
        t_prev, n_prev = tiles[-2]
        n_last = tiles[-1][1]
        total = n_prev + n_last
        n1 = (total // 2 + 127) // 128 * 128
        n2 = total - n1
        tiles[-2] = (t_prev, n1)
        tiles[-1] = (t_prev + n1, n2)
    return tiles


def _build_nc(cap, has_b1, has_b2):
    nc = bacc.Bacc("TRN2", target_bir_lowering=False, num_devices=N_CORES)

    xgt_d = nc.dram_tensor("xgt", [P, DK, cap], BF16, kind="ExternalInput")
    w1_d = nc.dram_tensor("w1", [P, DK, H], BF16, kind="ExternalInput")
    w2_d = nc.dram_tensor("w2", [P, DK, H], BF16, kind="ExternalInput")
    wp_d = nc.dram_tensor("wp", [P, HK, D], BF16, kind="ExternalInput")
    b1_d = nc.dram_tensor("b1", [P, HK], F32, kind="ExternalInput")
    b2_d = nc.dram_tensor("b2", [P, HK], F32, kind="ExternalInput")
    yt_d = nc.dram_tensor("yt", [P, DK, cap], F32, kind="ExternalOutput")

    ttiles = _token_tiles(cap)

    with tile.TileContext(nc) as tc:
        with (
            tc.tile_pool(name="wres", bufs=1) as wres,
            tc.tile_pool(name="hsp", bufs=1) as hsp,
            tc.tile_pool(name="silup", bufs=3) as silup,
            tc.tile_pool(name="h1sp", bufs=3) as h1sp,
            tc.tile_pool(name="ytsb", bufs=3) as ytsb,
            tc.tile_pool(name="ps", bufs=2, space="PSUM") as ps,
            tc.tile_pool(name="ps2", bufs=2, space="PSUM") as ps2,
        ):
            # ---- resident tiles ----
            w1_t = wres.tile([P, DK, H], BF16)
            w2_t = wres.tile([P, DK, H], BF16)
            wp_t = wres.tile([P, HK, D], BF16)
            xgt_t = wres.tile([P, DK, cap], BF16)
            b1_t = wres.tile([P, HK], F32)
            b2_t = wres.tile([P, HK], F32)
            # DMA issue costs ~0.6us/instr on a sequencer; spread the
            # critical first-tile loads across four sequencers so the PE can
            # start as early as possible, then stream the rest on sync/gpsimd.
            issuers = [nc.sync, nc.gpsimd, nc.scalar]
            crit = []
            t0_0, nt_0 = ttiles[0]
            HG = 4  # hk chunk for weight streaming
            hs0 = slice(0, HG * P)
            for dk in range(DK):
                crit.append((xgt_t[:, dk, t0_0:t0_0 + nt_0],
                             xgt_d[:, dk, t0_0:t0_0 + nt_0]))
            for dk in range(DK):
                crit.append((w2_t[:, dk, hs0], w2_d[:, dk, hs0]))
            for dk in range(DK):
                crit.append((w1_t[:, dk, hs0], w1_d[:, dk, hs0]))
            for i, (dst, srcap) in enumerate(crit):
                issuers[i % 3].dma_start(dst, srcap)
            rest = []
            for hk in range(HG, HK, HG):
                hs = slice(hk * P, (hk + HG) * P)
                for dk in range(DK):
                    rest.append((w2_t[:, dk, hs], w2_d[:, dk, hs]))
                    rest.append((w1_t[:, dk, hs], w1_d[:, dk, hs]))
            for (t0, nt) in ttiles[1:]:
                for dk in range(DK):
                    rest.append((xgt_t[:, dk, t0:t0 + nt],
                                 xgt_d[:, dk, t0:t0 + nt]))
            for hk in range(0, HK, HG):
                rest.append((wp_t[:, hk:hk + HG], wp_d[:, hk:hk + HG]))
            for i, (dst, srcap) in enumerate(rest):
                (nc.sync if i % 2 == 0 else nc.gpsimd).dma_start(dst, srcap)
            nc.sync.dma_start(b1_t[:], b1_d[:])
            nc.gpsimd.dma_start(b2_t[:], b2_d[:])

            for (t0, nt) in ttiles:
                hst = hsp.tile([P, HK, 512], BF16, name="hst")
                # ---- first GEMMs + SwiGLU ----
                for hk in range(HK):
                    h1p = ps.tile([P, 512], F32, name="h1p")
                    h2p = ps.tile([P, 512], F32, name="h2p")
                    for dk in range(DK):
                        nc.tensor.matmul(
                            h2p[:, :nt],
                            w2_t[:, dk, hk * P:(hk + 1) * P],
                            xgt_t[:, dk, t0:t0 + nt],
                            start=(dk == 0), stop=(dk == DK - 1),
                        )
                    for dk in range(DK):
                        nc.tensor.matmul(
                            h1p[:, :nt],
                            w1_t[:, dk, hk * P:(hk + 1) * P],
                            xgt_t[:, dk, t0:t0 + nt],
                            start=(dk == 0), stop=(dk == DK - 1),
                        )
                    s_t = silup.tile([P, 512], F32, name="s_t")
                    nc.scalar.activation(
                        s_t[:, :nt], h2p[:, :nt],
                        mybir.ActivationFunctionType.Silu,
                        bias=(b2_t[:, hk:hk + 1] if has_b2 else 0.0),
                    )
                    if has_b1:
                        h1s = h1sp.tile([P, 512], F32, name="h1s")
                        nc.vector.tensor_scalar_add(
                            h1s[:, :nt], h1p[:, :nt], b1_t[:, hk:hk + 1]
                        )
                        mul_in = h1s
                    else:
                        mul_in = h1p
                    nc.vector.tensor_mul(
                        out=hst[:, hk, :nt], in0=mul_in[:, :nt], in1=s_t[:, :nt]
                    )

                # ---- second GEMM, store transposed; host finishes ----
                for do in range(DK):
                    yp = ps2.tile([P, 512], F32, name="yp")
                    for hk in range(HK):
                        nc.tensor.matmul(
                            yp[:, :nt],
                            wp_t[:, hk, do * P:(do + 1) * P],
                            hst[:, hk, :nt],
                            start=(hk == 0), stop=(hk == HK - 1),
                        )
                    yo = ytsb.tile([P, 512], F32, name="yo")
                    nc.vector.tensor_copy(yo[:, :nt], yp[:, :nt])
                    nc.sync.dma_start(yt_d[:, do, t0:t0 + nt], yo[:, :nt])

    nc.compile()
    return nc


def kernel(x, gate_w, noise_weight, noise, w1, b1, w2, b2, wp, bp):
    global LAST_EXEC_NS, LAST_TRACE

    x = np.asarray(x, dtype=np.float32)
    gate_w = np.asarray(gate_w, dtype=np.float32)
    noise_weight = np.asarray(noise_weight, dtype=np.float32)
    noise = np.asarray(noise, dtype=np.float32)
    w1 = np.asarray(w1, dtype=np.float32)
    b1 = np.asarray(b1, dtype=np.float32)
    w2 = np.asarray(w2, dtype=np.float32)
    b2 = np.asarray(b2, dtype=np.float32)
    wp = np.asarray(wp, dtype=np.float32)
    bp = np.asarray(bp, dtype=np.float32)

    T = B * S
    xf = x.reshape(T, D)

    # ---------- host routing (control plane, fp32) ----------
    logits = xf @ gate_w.T                                   # [T, E]
    ln = logits
    if np.any(noise_weight):
        ln = logits + noise.reshape(T, E) * (NOISY_STD * noise_weight)
    part = np.partition(ln, E - 2, axis=1)
    t2 = part[:, E - 2]
    t1 = part[:, E - 1]
    mask = ln >= t2[:, None]                                 # top-2 set
    nsel = mask.sum(1)
    if np.any(nsel != K):  # tie fallback: exact top-k by sort
        order = np.argsort(-ln, axis=1, kind="stable")
        mask = np.zeros_like(mask)
        np.put_along_axis(mask, order[:, :K], True, axis=1)
    e_all = np.exp(ln - t1[:, None], dtype=np.float32)
    denom = 1.0 + np.exp(t2 - t1, dtype=np.float32)
    gates = np.where(mask, e_all / denom[:, None], 0.0).astype(np.float32)

    # load-balance loss from clean logits (full softmax)
    lmax = logits.max(1, keepdims=True)
    sm = np.exp(logits - lmax, dtype=np.float32)
    sm /= sm.sum(1, keepdims=True)
    gwm = sm.mean(0, dtype=np.float32)
    lb = np.float32(np.mean((gwm - 1.0 / E) ** 2, dtype=np.float32) * LB_SCALE)

    # ---------- dispatch: gather tokens per expert ----------
    idxs = [np.nonzero(mask[:, e])[0] for e in range(E)]
    counts = np.array([len(i) for i in idxs])
    cap = int(np.ceil(counts.max() / P) * P)

    has_b1 = bool(np.any(b1))
    has_b2 = bool(np.any(b2))

    in_maps = []
    for e in range(E):
        idx = idxs[e]
        n_e = len(idx)
        xg = np.zeros((cap, D), dtype=np.float32)
        xg[:n_e] = xf[idx]
        # [cap, D] -> [P, DK, cap] with element (p, dk, s) = xg[s, dk*128+p]
        xgt = np.ascontiguousarray(
            xg.reshape(cap, DK, P).transpose(2, 1, 0)
        ).astype(ml_dtypes.bfloat16)
        w1e = np.ascontiguousarray(
            w1[e].reshape(DK, P, H).transpose(1, 0, 2)
        ).astype(ml_dtypes.bfloat16)                          # [P, DK, H]
        w2e = np.ascontiguousarray(
            w2[e].reshape(DK, P, H).transpose(1, 0, 2)
        ).astype(ml_dtypes.bfloat16)
        wpe = np.ascontiguousarray(
            wp[e].reshape(HK, P, D).transpose(1, 0, 2)
        ).astype(ml_dtypes.bfloat16)                          # [P, HK, D]
        in_maps.append({
            "xgt": xgt, "w1": w1e, "w2": w2e, "wp": wpe,
            "b1": np.ascontiguousarray(b1[e].reshape(HK, P).T),
            "b2": np.ascontiguousarray(b2[e].reshape(HK, P).T),
        })

    # ---------- device: grouped SwiGLU GEMMs on 8 cores ----------
    key = (cap, has_b1, has_b2)
    if key not in _NC_CACHE:
        _NC_CACHE[key] = _build_nc(cap, has_b1, has_b2)
    nc = _NC_CACHE[key]
    trace = os.environ.get("MOE_TRACE") == "1"
    trace_all = os.environ.get("MOE_TRACE_ALL") == "1"
    out = run_bass_kernel_spmd(
        nc, in_maps, core_ids=list(range(N_CORES)), trace=trace or trace_all,
        trace_cores=list(range(N_CORES)) if trace_all else None,
    )
    global LAST_PER_CORE
    LAST_EXEC_NS = out.exec_time_ns
    LAST_PER_CORE = (out.mean_exec_time_ns, out.max_exec_time_core_id)
    LAST_TRACE = out.instructions_and_trace[1] if out.instructions_and_trace else None

    # ---------- combine on host ----------
    # Per-expert output yt [P, DK, cap] -> token-major [cap, D], + bp, * gate.
    # Each token has exactly K=2 contributions; gather-sum them.
    allout = np.empty((E * cap, D), dtype=np.float32)
    g_all = np.zeros(E * cap, dtype=np.float32)
    for e in range(E):
        yt = out.results[e]["yt"]                             # [P, DK, cap]
        allout[e * cap:(e + 1) * cap] = (
            yt.transpose(2, 1, 0).reshape(cap, D) + bp[e]
        )
        g_all[e * cap:e * cap + len(idxs[e])] = gates[idxs[e], e]
    allout *= g_all[:, None]

    pos = np.zeros((T, K), dtype=np.int64)
    cnt = np.zeros(T, dtype=np.int64)
    for e in range(E):
        idx = idxs[e]
        pos[idx, cnt[idx]] = e * cap + np.arange(len(idx))
        cnt[idx] += 1
    assert np.all(cnt == K)
    y = allout[pos[:, 0]] + allout[pos[:, 1]]
    return y.reshape(B, S, D).astype(np.float32), lb
